# revision 1
# baseline (speedup 1.0000x reference)
"""ConvPDDecoder Trainium2 kernel.

reference computation (B=4, N=2048, M=4096, D=1, C=4, K=32):
    d2[b,m,n] = (xz[m] - x[b,n])^2
    w = exp(-0.5*d2/exp(log_scale))            [B,M,N]
    zt[b,c,n,k] = sum_m w[b,m,n] z[b,c,m,k]    [B,C,N,K]
    out[b,c,n,m'] = sum_k zt[n,k] zt[m',k]     [B,C,N,N]
    returns (xz, out)

Sharding: 8 cores = 4 batches x 2 channel-pairs. Each core computes the
full RBF weight field for its batch (duplicated across the pair) and the
[2,N,N] output block for its two channels.

Device math uses the factorization (r = 1/sqrt(2*exp(log_scale))):
    E'[m,n] = exp(xs[n]*(2*xzs[m]) - xzs[m]^2),  xzs = xz*r, xs = x*r
            = w[m,n] * exp(xs[n]^2)
so one scalar-engine activation (per-partition scale/bias) produces each
[128,N] weight tile; the n-dependent correction v[n] = exp(-xs[n]^2) is
applied once to ztT after the PSUM accumulation.

Matmuls run in float32r (TF32-like, full PE rate at free dim >= 256).
"""

import numpy as np

B, N, M, C, K = 4, 2048, 4096, 4, 32
P = 128           # partitions
MCH = M // P      # 32 m-chunks
NSL = N // 512    # 4 n-slices of 512

_cache = {}


# --------------------------------------------------------------------------
# Workaround for this container's walrus build: it rejects instructions
# carrying more than one sync-wait command. Tile's scheduler attaches
# several waits to one instruction; split the extras onto fresh
# single-wait NoOps inserted just before, on the same engine stream.
def _split_multi_waits(nc):
    import concourse.mybir as mybir

    n_split = 0
    for bass_bb in nc.bb_map.values():
        bb = bass_bb.bb
        insts = list(bb.instructions)
        if not any(
            i.sync_info is not None and len(i.sync_info.on_wait) > 1 for i in insts
        ):
            continue
        newl = []
        for inst in insts:
            si = inst.sync_info
            if si is not None and len(si.on_wait) > 1:
                waits = list(si.on_wait)
                for w in waits[:-1]:
                    nop = mybir.InstNoOp(
                        name=nc.get_next_instruction_name(), ins=[], outs=[]
                    )
                    nop.engine = inst.engine
                    nop.sync_info = mybir.SyncInfo(on_wait=[w], on_update=[])
                    nc.register_instruction(nop)
                    newl.append(nop)
                    n_split += 1
                si.on_wait = [waits[-1]]
            newl.append(inst)
        bb.instructions = newl
    return n_split


def _build_nc():
    import concourse.bass as bass
    import concourse.mybir as mybir
    from concourse.tile import TileContext

    f32 = mybir.dt.float32
    f32r = mybir.dt.float32r
    Exp = mybir.ActivationFunctionType.Exp

    nc = bass.Bass()
    sc_d = nc.dram_tensor("sc", [P, MCH], f32, kind="ExternalInput")
    bi_d = nc.dram_tensor("bi", [P, MCH], f32, kind="ExternalInput")
    xs_d = nc.dram_tensor("xs", [N], f32, kind="ExternalInput")
    v_d = nc.dram_tensor("v", [N], f32, kind="ExternalInput")
    z_d = nc.dram_tensor("z", [P, MCH * 64], f32, kind="ExternalInput")
    out_d = nc.dram_tensor("out", [2, N, N], f32, kind="ExternalOutput")

    with TileContext(nc) as tc:
        with tc.tile_pool(name="consts", bufs=1) as consts:
            sc_t = consts.tile([P, MCH], f32)
            nc.sync.dma_start(sc_t[:], sc_d[:])
            bi_t = consts.tile([P, MCH], f32)
            nc.sync.dma_start(bi_t[:], bi_d[:])
            xs_b = consts.tile([P, N], f32)
            nc.gpsimd.dma_start(xs_b[:], xs_d[None, :].to_broadcast((P, N)))
            v_b = consts.tile([P, N], f32)
            nc.gpsimd.dma_start(v_b[:], v_d[None, :].to_broadcast((P, N)))
            z_r = consts.tile([P, MCH * 64], f32r)  # cast to f32r during DMA
            nc.gpsimd.dma_start(z_r[:], z_d[:])
            ztT = consts.tile([64, N], f32r)

            # ---- phase A: ztT[ck, n] = v[n] * sum_m z[m,ck] E'[m,n] ----
            with (
                tc.tile_pool(name="zt_ps", bufs=1, space="PSUM") as zt_pool,
                tc.tile_pool(name="epool", bufs=3) as epool,
            ):
                zt_ps = zt_pool.tile([64, N], f32)
                for mj in range(MCH):
                    e = epool.tile([P, N], f32r)
                    nc.scalar.activation(
                        e[:], xs_b[:], Exp,
                        bias=bi_t[:, mj:mj + 1], scale=sc_t[:, mj:mj + 1],
                    )
                    for ns in range(NSL):
                        nc.tensor.matmul(
                            zt_ps[:, ns * 512:(ns + 1) * 512],
                            z_r[:, mj * 64:(mj + 1) * 64],
                            e[:, ns * 512:(ns + 1) * 512],
                            start=(mj == 0), stop=(mj == MCH - 1),
                        )
                nc.vector.tensor_mul(ztT[:], zt_ps[:], v_b[:64, :])

            # ---- phase B: out[c', nr, :] = ztT[c']^T @ ztT[c'] ----
            with (
                tc.tile_pool(name="out_ps", bufs=2, space="PSUM") as out_pool,
                tc.tile_pool(name="stage", bufs=4) as spool,
            ):
                it = 0
                for ci in range(2):
                    for nr in range(N // P):
                        po = out_pool.tile([P, N], f32)
                        for ms in range(NSL):
                            nc.tensor.matmul(
                                po[:, ms * 512:(ms + 1) * 512],
                                ztT[ci * 32:(ci + 1) * 32, nr * P:(nr + 1) * P],
                                ztT[ci * 32:(ci + 1) * 32, ms * 512:(ms + 1) * 512],
                                start=True, stop=True,
                            )
                        st = spool.tile([P, N], f32)
                        # ACT takes every 8th copy; DVE the rest
                        if it % 8 == 7:
                            nc.scalar.copy(st[:], po[:])
                        else:
                            nc.vector.tensor_copy(st[:], po[:])
                        nc.sync.dma_start(out_d[ci, nr * P:(nr + 1) * P, :], st[:])
                        it += 1

    _split_multi_waits(nc)
    return nc


def _prep_inputs(xz, x, z, log_scale):
    """Host-side prep: tiny O(M+N) trig plus one z transpose."""
    s = np.exp(np.float32(log_scale[0]))
    r = np.float32(1.0 / np.sqrt(2.0 * s))
    xzs = (xz[:, 0] * r).astype(np.float32)          # [M]
    sc = np.ascontiguousarray((2.0 * xzs).reshape(MCH, P).T)   # [P, MCH]
    bi = np.ascontiguousarray((-(xzs * xzs)).reshape(MCH, P).T)

    in_maps = []
    for core in range(8):
        b, cp = core // 2, core % 2
        xs = (x[b, :, 0] * r).astype(np.float32)     # [N]
        v = np.exp(-(xs * xs)).astype(np.float32)
        # z_core[p, mj*64 + c'*32 + k] = z[b, 2cp+c', mj*128+p, k]
        zz = z[b, 2 * cp:2 * cp + 2]                  # [2, M, K]
        zz = zz.reshape(2, MCH, P, K).transpose(2, 1, 0, 3).reshape(P, MCH * 64)
        in_maps.append({
            "sc": sc, "bi": bi, "xs": xs, "v": v,
            "z": np.ascontiguousarray(zz),
        })
    return in_maps


def kernel(xz, x, z, log_scale):
    from concourse.bass_utils import run_bass_kernel_spmd

    if "nc" not in _cache:
        _cache["nc"] = _build_nc()
    nc = _cache["nc"]

    in_maps = _prep_inputs(xz, x, z, log_scale)
    res = run_bass_kernel_spmd(nc, in_maps, core_ids=list(range(8)))

    out = np.empty((B, C, N, N), dtype=np.float32)
    for core in range(8):
        b, cp = core // 2, core % 2
        out[b, 2 * cp:2 * cp + 2] = res.results[core]["out"]
    return (xz, out)


# revision 3
# speedup vs baseline: 193.1523x; 193.1523x over previous
"""ConvPDDecoder Trainium2 kernel.

reference computation (B=4, N=2048, M=4096, D=1, C=4, K=32):
    d2[b,m,n] = (xz[m] - x[b,n])^2
    w = exp(-0.5*d2/exp(log_scale))            [B,M,N]
    zt[b,c,n,k] = sum_m w[b,m,n] z[b,c,m,k]    [B,C,N,K]
    out[b,c,n,m'] = sum_k zt[n,k] zt[m',k]     [B,C,N,N]
    returns (xz, out)

Sharding: 8 cores = 4 batches x 2 channel-pairs. Each core computes the
full RBF weight field for its batch (duplicated across the pair) and the
[2,N,N] output block for its two channels.

Device math uses the factorization (r = 1/sqrt(2*exp(log_scale))):
    E'[m,n] = exp(xs[n]*(2*xzs[m]) - xzs[m]^2),  xzs = xz*r, xs = x*r
            = w[m,n] * exp(xs[n]^2)
so one scalar-engine activation (per-partition scale/bias) produces each
[128,N] weight tile; the n-dependent correction v[n] = exp(-xs[n]^2) is
applied once to ztT after the PSUM accumulation.

Matmuls run in float32r (TF32-like, full PE rate at free dim >= 256).
"""

import numpy as np

B, N, M, C, K = 4, 2048, 4096, 4, 32
P = 128           # partitions
MCH = M // P      # 32 m-chunks
NSL = N // 512    # 4 n-slices of 512

_cache = {}


# --------------------------------------------------------------------------
# Workaround for this container's walrus build: it rejects instructions
# carrying more than one sync-wait command. Tile's scheduler attaches
# several waits to one instruction; split the extras onto fresh
# single-wait NoOps inserted just before, on the same engine stream.
def _split_multi_waits(nc):
    import concourse.mybir as mybir

    n_split = 0
    for bass_bb in nc.bb_map.values():
        bb = bass_bb.bb
        insts = list(bb.instructions)
        if not any(
            i.sync_info is not None and len(i.sync_info.on_wait) > 1 for i in insts
        ):
            continue
        newl = []
        for inst in insts:
            si = inst.sync_info
            if si is not None and len(si.on_wait) > 1:
                waits = list(si.on_wait)
                for w in waits[:-1]:
                    nop = mybir.InstNoOp(
                        name=nc.get_next_instruction_name(), ins=[], outs=[]
                    )
                    nop.engine = inst.engine
                    nop.sync_info = mybir.SyncInfo(on_wait=[w], on_update=[])
                    nc.register_instruction(nop)
                    newl.append(nop)
                    n_split += 1
                si.on_wait = [waits[-1]]
            newl.append(inst)
        bb.instructions = newl
    return n_split


def _build_nc(repeat=1):
    import concourse.bass as bass
    import concourse.mybir as mybir
    from concourse.tile import TileContext

    f32 = mybir.dt.float32
    f32r = mybir.dt.float32r
    Exp = mybir.ActivationFunctionType.Exp

    nc = bass.Bass()
    sc_d = nc.dram_tensor("sc", [P, MCH], f32, kind="ExternalInput")
    bi_d = nc.dram_tensor("bi", [P, MCH], f32, kind="ExternalInput")
    xs_d = nc.dram_tensor("xs", [N], f32, kind="ExternalInput")
    v_d = nc.dram_tensor("v", [N], f32, kind="ExternalInput")
    z_d = nc.dram_tensor("z", [P, MCH * 64], f32, kind="ExternalInput")
    out_d = nc.dram_tensor("out", [2, N, N], f32, kind="ExternalOutput")

    with TileContext(nc) as tc:
        with tc.tile_pool(name="consts", bufs=1) as consts:
            sc_t = consts.tile([P, MCH], f32)
            nc.sync.dma_start(sc_t[:], sc_d[:])
            bi_t = consts.tile([P, MCH], f32)
            nc.sync.dma_start(bi_t[:], bi_d[:])
            xs_b = consts.tile([P, N], f32)
            nc.gpsimd.dma_start(xs_b[:], xs_d[None, :].to_broadcast((P, N)))
            v_b = consts.tile([P, N], f32)
            nc.gpsimd.dma_start(v_b[:], v_d[None, :].to_broadcast((P, N)))
            z_r = consts.tile([P, MCH * 64], f32r)  # cast to f32r during DMA
            nc.gpsimd.dma_start(z_r[:], z_d[:])

            for _rep in range(repeat):
                ztT = consts.tile([64, N], f32r, tag="ztT")

                # ---- phase A: ztT[ck,n] = v[n]*sum_m z[m,ck] E'[m,n] ----
                with (
                    tc.tile_pool(name="zt_ps", bufs=1, space="PSUM") as zt_pool,
                    tc.tile_pool(name="epool", bufs=3) as epool,
                ):
                    zt_ps = zt_pool.tile([64, N], f32)
                    for mj in range(MCH):
                        e = epool.tile([P, N], f32r)
                        nc.scalar.activation(
                            e[:], xs_b[:], Exp,
                            bias=bi_t[:, mj:mj + 1], scale=sc_t[:, mj:mj + 1],
                        )
                        for ns in range(NSL):
                            nc.tensor.matmul(
                                zt_ps[:, ns * 512:(ns + 1) * 512],
                                z_r[:, mj * 64:(mj + 1) * 64],
                                e[:, ns * 512:(ns + 1) * 512],
                                start=(mj == 0), stop=(mj == MCH - 1),
                            )
                    nc.vector.tensor_mul(ztT[:], zt_ps[:], v_b[:64, :])

                # ---- phase B: out[c', nr, :] = ztT[c']^T @ ztT[c'] ----
                with (
                    tc.tile_pool(name="out_ps", bufs=2, space="PSUM") as out_pool,
                    tc.tile_pool(name="stage", bufs=4) as spool,
                ):
                    it = 0
                    for ci in range(2):
                        for nr in range(N // P):
                            po = out_pool.tile([P, N], f32)
                            for ms in range(NSL):
                                nc.tensor.matmul(
                                    po[:, ms * 512:(ms + 1) * 512],
                                    ztT[ci * 32:(ci + 1) * 32, nr * P:(nr + 1) * P],
                                    ztT[ci * 32:(ci + 1) * 32, ms * 512:(ms + 1) * 512],
                                    start=True, stop=True,
                                )
                            st = spool.tile([P, N], f32)
                            # ACT takes every 8th copy; DVE the rest
                            if it % 8 == 7:
                                nc.scalar.copy(st[:], po[:])
                            else:
                                nc.vector.tensor_copy(st[:], po[:])
                            nc.sync.dma_start(
                                out_d[ci, nr * P:(nr + 1) * P, :], st[:]
                            )
                            it += 1

    _split_multi_waits(nc)
    return nc


def _make_runner(nc, n_cores=8):
    """Cached PJRT runner: jit once, reuse across calls.

    Returns run(in_maps) -> list of per-core {name: np.ndarray}.
    """
    import jax
    import jax.numpy as jnp
    from jax.sharding import Mesh, PartitionSpec
    from jax.experimental.shard_map import shard_map
    import concourse.mybir as mybir
    from concourse import bass2jax

    bass2jax.install_neuronx_cc_hook()

    partition_name = (
        nc.partition_id_tensor.name if nc.partition_id_tensor else None
    )
    in_names, out_names, out_avals, zero_outs = [], [], [], []
    for alloc in nc.m.functions[0].allocations:
        if not isinstance(alloc, mybir.MemoryLocationSet):
            continue
        name = alloc.memorylocations[0].name
        if alloc.kind == "ExternalInput":
            if name != partition_name:
                in_names.append(name)
        elif alloc.kind == "ExternalOutput":
            out_names.append(name)
            shape = tuple(alloc.tensor_shape)
            dtype = mybir.dt.np(alloc.dtype)
            out_avals.append(jax.core.ShapedArray(shape, dtype))
            zero_outs.append(np.zeros(shape, dtype))
    n_params = len(in_names)
    all_in = tuple(in_names + out_names)
    if partition_name is not None:
        all_in = all_in + (partition_name,)

    def _body(*args):
        operands = list(args)
        if partition_name is not None:
            operands.append(bass2jax.partition_id_tensor())
        outs = bass2jax._bass_exec_p.bind(
            *operands,
            out_avals=tuple(out_avals),
            in_names=all_in,
            out_names=tuple(out_names),
            lowering_input_output_aliases=(),
            sim_require_finite=True,
            sim_require_nnan=True,
            nc=nc,
        )
        return tuple(outs)

    devices = jax.devices()[:n_cores]
    mesh = Mesh(np.asarray(devices), ("core",))
    nio = n_params + len(out_names)
    sharded = jax.jit(
        shard_map(
            _body,
            mesh=mesh,
            in_specs=(PartitionSpec("core"),) * nio,
            out_specs=(PartitionSpec("core"),) * len(out_names),
            check_rep=False,
        ),
        keep_unused=True,
    )

    concat_zeros = [
        np.zeros((n_cores * z.shape[0], *z.shape[1:]), z.dtype) for z in zero_outs
    ]

    def place(in_maps):
        """Concatenate per-core inputs and place on devices (+ zero outs)."""
        concat_in = [
            np.concatenate([np.asarray(m[name]) for m in in_maps], axis=0)
            for name in in_names
        ]
        return concat_in + concat_zeros

    def run_placed(args):
        return sharded(*args)

    def fetch(out_arrs, n_cores=n_cores):
        return [
            {
                name: np.asarray(out_arrs[i]).reshape(
                    n_cores, *out_avals[i].shape
                )[c]
                for i, name in enumerate(out_names)
            }
            for c in range(n_cores)
        ]

    def run(in_maps):
        return fetch(run_placed(place(in_maps)))

    run.place = place
    run.run_placed = run_placed
    run.fetch = fetch
    return run


def _get_runner(repeat=1):
    key = ("runner", repeat)
    if key not in _cache:
        _cache[key] = _make_runner(_build_nc(repeat=repeat))
    return _cache[key]


def _prep_inputs(xz, x, z, log_scale):
    """Host-side prep: tiny O(M+N) math plus one z transpose."""
    s = np.exp(np.float32(log_scale[0]))
    r = np.float32(1.0 / np.sqrt(2.0 * s))
    xzs = (xz[:, 0] * r).astype(np.float32)          # [M]
    sc = np.ascontiguousarray((2.0 * xzs).reshape(MCH, P).T)   # [P, MCH]
    bi = np.ascontiguousarray((-(xzs * xzs)).reshape(MCH, P).T)

    in_maps = []
    for core in range(8):
        b, cp = core // 2, core % 2
        xs = (x[b, :, 0] * r).astype(np.float32)     # [N]
        v = np.exp(-(xs * xs)).astype(np.float32)
        # z_core[p, mj*64 + c'*32 + k] = z[b, 2cp+c', mj*128+p, k]
        zz = z[b, 2 * cp:2 * cp + 2]                  # [2, M, K]
        zz = zz.reshape(2, MCH, P, K).transpose(2, 1, 0, 3).reshape(P, MCH * 64)
        in_maps.append({
            "sc": sc, "bi": bi, "xs": xs, "v": v,
            "z": np.ascontiguousarray(zz),
        })
    return in_maps


def kernel(xz, x, z, log_scale):
    run = _get_runner()
    in_maps = _prep_inputs(xz, x, z, log_scale)
    results = run(in_maps)

    out = np.empty((B, C, N, N), dtype=np.float32)
    for core in range(8):
        b, cp = core // 2, core % 2
        out[b, 2 * cp:2 * cp + 2] = results[core]["out"]
    return (xz, out)
